# revision 17
# baseline (speedup 1.0000x reference)
"""Bass/Trainium2 kernel for nn_MetaSpace_68822555951687.

Gaussian keypoint pooling over two feature-map levels + per-token fusion
network (LN/MLP projection + 2-token gated attention), data-parallel over
batch B=32 across 8 NeuronCores.

Strategy:
- Host: transpose feature maps to channel-last (pure layout change), shard
  along B, precompute gather row indices and clip-merged separable Gaussian
  weights as block-diagonal matmul operands, pack all network weights into
  one [128, cols] image (with 1/sqrt(d) folded into wq and the attention
  s-mean 0.5 folded into wo).
- Device (SPMD, one program): one batched indirect DMA gather per level
  (each descriptor = one pixel's C channels, contiguous), pooling as
  block-diagonal matmuls on the TensorEngine (PE applies the Gaussian
  weights directly), then the token network with float32r matmuls.
"""

import math
from contextlib import ExitStack

import numpy as np

import concourse.bass as bass
import concourse.tile as tile
from concourse import bacc, mybir
from concourse.masks import make_identity

# ---------------------------------------------------------------- constants
ORIG_H, ORIG_W = 256, 192
NUM_HEADS = 8
KSZ, SIGMA = 5, 2.0
B, N = 32, 17
N_CORES = 8
B_LOC = B // N_CORES       # 4
T = B_LOC * N              # 68 tokens per core
GSZ = 5                    # windows per pooling matmul group (5*25=125<=128)
NG = (T + GSZ - 1) // GSZ  # 14 groups
D_HEAD_0, D_HEAD_1 = 256 // NUM_HEADS, 512 // NUM_HEADS

LEVELS = [
    dict(C=256, H=96, W=72),
    dict(C=512, H=48, W=36),
]

F32 = mybir.dt.float32
F32R = mybir.dt.float32r
I32 = mybir.dt.int32
I16 = mybir.dt.int16
ALU = mybir.AluOpType
ACTF = mybir.ActivationFunctionType

MM_DT = F32R  # matmul operand dtype (bitcast view of f32 data)


def _mm(ap):
    return ap.bitcast(MM_DT)


# ------------------------------------------------------------ params layout
class Layout:
    """Column layout of the packed [128, cols] params image."""

    def __init__(self):
        self.cols = 0
        self.blocks = {}

    def add(self, name, ncols):
        self.blocks[name] = (self.cols, ncols)
        self.cols += ncols

    def sl(self, name):
        off, n = self.blocks[name]
        return slice(off, off + n)


def make_layout():
    lay = Layout()
    for l, lv in enumerate(LEVELS):
        C = lv["C"]
        for t in range(2 * C // 128):
            lay.add(f"w1_{l}_{t}", C)
        for t in range(C // 128):
            lay.add(f"w2_{l}_{t}", C)
        for t in range(C // 128):
            lay.add(f"wqkv_{l}_{t}", 3 * C)
        for t in range(C // 128):
            lay.add(f"wo_{l}_{t}", C)
        for t in range(C // 128):
            lay.add(f"wg_{l}_{t}", NUM_HEADS)
        for t in range(C // 128):
            lay.add(f"metaT_{l}_{t}", T)
        lay.add(f"g_{l}", C)
        lay.add(f"beta_{l}", C)
    return lay


def make_bias_layout():
    lay = Layout()
    lay.add("ones", T)
    for l, lv in enumerate(LEVELS):
        C = lv["C"]
        lay.add(f"b1_{l}", C)
        lay.add(f"b2_{l}", C)
        lay.add(f"bqkv_{l}", 3 * C)
        lay.add(f"bo_{l}", C)
        lay.add(f"bg_{l}", NUM_HEADS)
    return lay


LAY = make_layout()
BLAY = make_bias_layout()

# split point for the two params DMA chunks: everything through level-0
# blocks first, level-1 weights second
_PSPLIT = LAY.blocks["w1_1_0"][0]


# ------------------------------------------------------------- host helpers
def _gauss_1d():
    off = np.arange(KSZ, dtype=np.float32) - (KSZ // 2)
    return np.exp(-(off ** 2) / (2.0 * np.float32(SIGMA) ** 2))


def _window_geometry(kpts, H, W):
    """Per-window clipped origin + merged separable weights.

    kpts: [nw, 2] float32 (x, y) in original image coords.
    Returns y0, x0 (int32 [nw]) and wy, wx (float32 [nw, 5]) such that
    sum_{dy,dx} wy[dy]*wx[dx]*fm[y0+dy, x0+dx] equals the reference's
    clipped Gaussian pooling (including the global 1/sum normalization).
    """
    nw = kpts.shape[0]
    g1 = _gauss_1d()
    norm = g1.sum()  # g.sum() over 5x5 = (norm)^2
    sx = np.float32(W) / np.float32(ORIG_W)
    sy = np.float32(H) / np.float32(ORIG_H)
    rx = np.round(kpts[:, 0].astype(np.float32) * sx).astype(np.int64)
    ry = np.round(kpts[:, 1].astype(np.float32) * sy).astype(np.int64)
    x0 = np.clip(rx - 2, 0, W - KSZ)
    y0 = np.clip(ry - 2, 0, H - KSZ)
    wx = np.zeros((nw, KSZ), np.float32)
    wy = np.zeros((nw, KSZ), np.float32)
    for o in range(KSZ):
        cx = np.clip(rx - 2 + o, 0, W - 1) - x0   # in [0, 4]
        cy = np.clip(ry - 2 + o, 0, H - 1) - y0
        np.add.at(wx, (np.arange(nw), cx), g1[o])
        np.add.at(wy, (np.arange(nw), cy), g1[o])
    return (
        y0.astype(np.int32),
        x0.astype(np.int32),
        wy / norm,
        wx / norm,
    )


def _pack_ktiles(lay, params, name, w):
    """Pack weight w [K, N] as K/128 column blocks of the params image."""
    K = w.shape[0]
    for t in range(K // 128):
        params[:, lay.sl(f"{name}_{t}")] = w[t * 128:(t + 1) * 128, :]


def prepare_core_inputs(inputs):
    """Full inputs -> list of per-core in_maps (host-side prep only)."""
    kpts = np.asarray(inputs["keypoints"], np.float32)

    # shared (batch-independent) params image
    params = np.zeros((128, LAY.cols), np.float32)
    biasrow = np.zeros((1, BLAY.cols), np.float32)
    biasrow[0, BLAY.sl("ones")] = 1.0
    for l, lv in enumerate(LEVELS):
        C = lv["C"]
        d = C // NUM_HEADS
        qs = np.float32(1.0 / math.sqrt(d))
        wqkv = np.array(inputs[f"a{l}_wqkv"], np.float32).copy()
        bqkv = np.array(inputs[f"a{l}_bqkv"], np.float32).copy()
        wqkv[:, :C] *= qs
        bqkv[:C] *= qs
        wo = np.asarray(inputs[f"a{l}_wo"], np.float32) * np.float32(0.5)
        _pack_ktiles(LAY, params, f"w1_{l}", np.asarray(inputs[f"p{l}_w1"], np.float32))
        _pack_ktiles(LAY, params, f"w2_{l}", np.asarray(inputs[f"p{l}_w2"], np.float32))
        _pack_ktiles(LAY, params, f"wqkv_{l}", wqkv)
        _pack_ktiles(LAY, params, f"wo_{l}", wo)
        _pack_ktiles(LAY, params, f"wg_{l}", np.asarray(inputs[f"a{l}_wg"], np.float32))
        msT = np.asarray(inputs[f"ms{l}"], np.float32).T  # [C, N]
        metaT = np.tile(msT, (1, B_LOC))                  # [C, T], token = b*N+n
        _pack_ktiles(LAY, params, f"metaT_{l}", metaT.reshape(C, T))
        params[:, LAY.sl(f"g_{l}")] = np.asarray(inputs[f"p{l}_g"], np.float32)[None, :]
        params[:, LAY.sl(f"beta_{l}")] = np.asarray(inputs[f"p{l}_beta"], np.float32)[None, :]
        biasrow[0, BLAY.sl(f"b1_{l}")] = np.asarray(inputs[f"p{l}_b1"], np.float32)
        biasrow[0, BLAY.sl(f"b2_{l}")] = np.asarray(inputs[f"p{l}_b2"], np.float32)
        biasrow[0, BLAY.sl(f"bqkv_{l}")] = bqkv
        biasrow[0, BLAY.sl(f"bo_{l}")] = np.asarray(inputs[f"a{l}_bo"], np.float32)
        biasrow[0, BLAY.sl(f"bg_{l}")] = np.asarray(inputs[f"a{l}_bg"], np.float32)

    in_maps = []
    for core in range(N_CORES):
        b0 = core * B_LOC
        m = {"params": params, "biasrow": biasrow}
        idx = np.zeros((128, 2 * NG), np.int32)
        wblk_all = np.zeros((128, 2 * NG * GSZ), np.float32)
        for l, lv in enumerate(LEVELS):
            C, H, W = lv["C"], lv["H"], lv["W"]
            fm = np.asarray(inputs[f"fm{l}"], np.float32)[b0:b0 + B_LOC]
            fm_cl = np.ascontiguousarray(fm.transpose(0, 2, 3, 1))
            m[f"fm{l}"] = fm_cl.reshape(B_LOC * H * W, C)
            k = kpts[b0:b0 + B_LOC].reshape(T, 2)
            y0, x0, wy, wx = _window_geometry(k, H, W)
            # gather indices: partition p = w5*25 + dy*5 + dx, column = group
            wblk = np.zeros((128, NG * GSZ), np.float32)
            for g in range(NG):
                for w5 in range(GSZ):
                    w = g * GSZ + w5
                    if w >= T:
                        continue
                    bw = w // N
                    for dy in range(KSZ):
                        row = (bw * H + y0[w] + dy) * W + x0[w]
                        for dx in range(KSZ):
                            p = w5 * 25 + dy * 5 + dx
                            idx[p, l * NG + g] = row + dx
                            wblk[p, g * GSZ + w5] = wy[w, dy] * wx[w, dx]
            wblk_all[:, l * NG * GSZ:(l + 1) * NG * GSZ] = wblk
        m["idx"] = idx
        m["wblk"] = wblk_all
        in_maps.append(m)
    return in_maps


# --------------------------------------------------------------- bass build
def build_program(loop=0):
    nc = bacc.Bacc("TRN2", target_bir_lowering=False, debug=False,
                   num_devices=N_CORES)

    fm_d = [
        nc.dram_tensor(f"fm{l}", [B_LOC * lv["H"] * lv["W"], lv["C"]], F32,
                       kind="ExternalInput")
        for l, lv in enumerate(LEVELS)
    ]
    params_d = nc.dram_tensor("params", [128, LAY.cols], F32R, kind="ExternalInput")
    biasrow_d = nc.dram_tensor("biasrow", [1, BLAY.cols], F32R, kind="ExternalInput")
    idx_d = nc.dram_tensor("idx", [128, 2 * NG], I32, kind="ExternalInput")
    wblk_d = nc.dram_tensor("wblk", [128, 2 * NG * GSZ], F32, kind="ExternalInput")
    f_d = [
        nc.dram_tensor(f"f{l}", [B_LOC, N, lv["C"]], F32, kind="ExternalOutput")
        for l, lv in enumerate(LEVELS)
    ]
    with tile.TileContext(nc) as tc:
        with ExitStack() as ctx:
            if loop:
                with tc.For_i(0, loop, 1):
                    with ExitStack() as ictx:
                        _build_body(ictx, tc, fm_d, params_d, biasrow_d, idx_d,
                                    wblk_d, f_d)
            else:
                _build_body(ctx, tc, fm_d, params_d, biasrow_d, idx_d, wblk_d,
                            f_d)

    nc.compile()
    return nc


def _build_body(ctx, tc, fm_d, params_d, biasrow_d, idx_d, wblk_d, f_d):
    nc = tc.nc

    const = ctx.enter_context(tc.tile_pool(name="const", bufs=1))
    gpool = ctx.enter_context(tc.tile_pool(name="gather", bufs=1))
    sb = ctx.enter_context(tc.tile_pool(name="work", bufs=1))
    sm = ctx.enter_context(tc.tile_pool(name="small", bufs=2))
    ps = ctx.enter_context(tc.tile_pool(name="psum", bufs=1, space="PSUM"))
    psF = ctx.enter_context(tc.tile_pool(name="psumF", bufs=2, space="PSUM"))
    ps2 = ctx.enter_context(tc.tile_pool(name="psum2", bufs=2, space="PSUM"))

    # ---- small constants
    idx_t = const.tile([128, 2 * NG], I32)
    nc.sync.dma_start(idx_t[:], idx_d.ap())
    wblk_t = const.tile([128, 2 * NG * GSZ], F32)
    nc.sync.dma_start(wblk_t[:], wblk_d.ap())
    biasrow_t = const.tile([1, BLAY.cols], F32R)
    nc.sync.dma_start(biasrow_t[:], biasrow_d.ap())
    eps_t = const.tile([T, 1], F32)
    nc.vector.memset(eps_t[:], 1e-5)
    ident = const.tile([128, 128], F32)
    make_identity(nc, ident[:])

    # ---- params image, split into two chunks so level-0 compute starts early
    params_t = const.tile([128, LAY.cols], F32R)
    nc.sync.dma_start(params_t[:, :_PSPLIT], params_d.ap()[:, :_PSPLIT])

    # ---- batched gathers (one indirect DMA per level)
    gt = []
    for l, lv in enumerate(LEVELS):
        C = lv["C"]
        g = gpool.tile([128, NG, C], F32, tag=f"G{l}")
        for grp in range(NG):
            nc.gpsimd.indirect_dma_start(
                out=g[:, grp, :],
                out_offset=None,
                in_=fm_d[l].ap(),
                in_offset=bass.IndirectOffsetOnAxis(
                    ap=idx_t[:, l * NG + grp:l * NG + grp + 1], axis=0),
            )
        gt.append(g)

    # second params chunk (level-1 weights)
    nc.sync.dma_start(params_t[:, _PSPLIT:], params_d.ap()[:, _PSPLIT:])

    def P(name):
        return params_t[:, LAY.sl(name)]

    def BR(name):
        return biasrow_t[:, BLAY.sl(name)]

    for l, lv in enumerate(LEVELS):
        C = lv["C"]
        nch = C // 128
        dh = C // NUM_HEADS

        # ---- pooling: kfT[ch] = G_chunk.T @ wblk  (c-major keypoint feats)
        wblk = wblk_t[:, l * NG * GSZ:(l + 1) * NG * GSZ]
        kfT = []
        for ch in range(nch):
            kfp = ps2.tile([128, NG * GSZ], F32, tag="kfpt")
            for g in range(NG):
                nc.tensor.matmul(
                    out=kfp[:, g * GSZ:(g + 1) * GSZ],
                    lhsT=gt[l][:, g, ch * 128:(ch + 1) * 128],
                    rhs=wblk[:, g * GSZ:(g + 1) * GSZ],
                    start=True, stop=True,
                )
            k = sm.tile([128, T], F32R, tag=f"kfT{l}_{ch}")
            nc.vector.tensor_copy(k[:], kfp[:, :T])
            kfT.append(k)

        metaT = [P(f"metaT_{l}_{ch}") for ch in range(nch)]

        def mm_acc(out_ap, lhsTs, rhs_of_ch, bias_ap, start=True, stop=True):
            """out += sum_ch lhsTs[ch].T @ rhs_of_ch(ch) + ones.T @ bias."""
            nmm = len(lhsTs)
            for i, lt in enumerate(lhsTs):
                nc.tensor.matmul(out=out_ap, lhsT=_mm(lt),
                                 rhs=_mm(rhs_of_ch(i)),
                                 start=start and i == 0, stop=False)
            nc.tensor.matmul(out=out_ap, lhsT=_mm(biasrow_t[:, BLAY.sl("ones")]),
                             rhs=_mm(bias_ap), start=False, stop=stop)

        # ---- h1 = concat(kf, meta) @ w1 + b1 ; LN ; relu
        h1 = ps.tile([T, C], F32, tag="h1")
        xt_all = kfT + metaT
        mm_acc(h1[:, :], xt_all,
               lambda i: P(f"w1_{l}_{i}"), BR(f"b1_{l}"))

        stats = sm.tile([T, 6], F32, tag="stats")
        nc.vector.bn_stats(out=stats[:], in_=h1[:, :])
        mv = sm.tile([T, 2], F32, tag="mv")
        nc.vector.bn_aggr(out=mv[:], in_=stats[:])
        rstd = sm.tile([T, 1], F32, tag="rstd")
        nc.scalar.activation(out=rstd[:], in_=mv[:, 1:2], func=ACTF.Sqrt,
                             bias=eps_t[:], scale=1.0)
        nc.vector.reciprocal(out=rstd[:], in_=rstd[:])
        xr = sb.tile([T, C], F32, tag=f"xr{l}")
        nc.vector.tensor_scalar(xr[:], h1[:, :], mv[:, 0:1], None, ALU.subtract)
        nc.vector.scalar_tensor_tensor(
            out=xr[:], in0=xr[:], scalar=rstd[:, 0:1], in1=P(f"g_{l}")[:T, :],
            op0=ALU.mult, op1=ALU.mult)
        nc.vector.tensor_tensor(out=xr[:], in0=xr[:],
                                in1=P(f"beta_{l}")[:T, :], op=ALU.add)
        nc.vector.tensor_scalar(xr[:], xr[:], 0.0, None, ALU.max)

        # transpose xr -> c-major for the w2 matmul
        xrT = []
        for ch in range(nch):
            tp = ps2.tile([128, T], F32, tag="kfpt")
            nc.tensor.transpose(out=tp[:], in_=xr[:, ch * 128:(ch + 1) * 128],
                                identity=ident[:T, :T])
            xt = sm.tile([128, T], F32R, tag=f"xrT{l}_{ch}")
            nc.vector.tensor_copy(xt[:], tp[:])
            xrT.append(xt)

        # ---- final accumulator: projected = relu(LN) @ w2 + b2 (+ attn out)
        F_ps = psF.tile([T, C], F32, tag="F")
        mm_acc(F_ps[:, :], xrT, lambda i: P(f"w2_{l}_{i}"), BR(f"b2_{l}"),
               start=True, stop=False)

        # ---- qkv for s=0 (kf) and s=1 (meta), staged PSUM -> SBUF
        qc = sb.tile([T, 2, C], F32, tag="qsb")
        kc = sb.tile([T, 2, C], F32, tag="ksb")
        vc = sb.tile([T, 2, C], F32, tag="vsb")
        gc = ps.tile([T, 2, NUM_HEADS], F32, tag="gc")
        for s, xT in ((0, kfT), (1, metaT)):
            for dst, c0 in ((qc, 0), (kc, C), (vc, 2 * C)):
                pp = ps2.tile([T, C], F32, tag="qkvp")
                mm_acc(pp[:, :], xT,
                       lambda i, c0=c0: P(f"wqkv_{l}_{i}")[:, c0:c0 + C],
                       BR(f"bqkv_{l}")[:, c0:c0 + C])
                nc.vector.tensor_copy(dst[:, s, :], pp[:, :])
            mm_acc(gc[:, s, :], xT, lambda i: P(f"wg_{l}_{i}"), BR(f"bg_{l}"))
        ga = sm.tile([T, 2, NUM_HEADS], F32, tag="ga")
        nc.scalar.activation(out=ga[:, :, :], in_=gc[:, :, :], func=ACTF.Sigmoid)

        # ---- attention over the 2-token axis (softmax == sigmoid of diff)
        prod = sb.tile([T, 2, 2, C], F32, tag=f"attbuf{l}")
        nc.vector.tensor_tensor(
            out=prod[:, :, :, :],
            in0=qc[:, :, :].unsqueeze(2).to_broadcast([T, 2, 2, C]),
            in1=kc[:, :, :].unsqueeze(1).to_broadcast([T, 2, 2, C]),
            op=ALU.mult)
        logit = sm.tile([T, 2, 2, NUM_HEADS], F32, tag="logit")
        nc.vector.tensor_reduce(
            out=logit[:, :, :, :],
            in_=prod[:, :, :, :].rearrange("p s t (h d) -> p s t h d", d=dh),
            axis=mybir.AxisListType.X, op=ALU.add)
        att = sm.tile([T, 2, 2, NUM_HEADS], F32, tag="att")
        # att[:, s, 1, :] = sigmoid(l_s1 - l_s0); att[:, s, 0, :] = 1 - that
        nc.vector.tensor_tensor(out=att[:, :, 1, :], in0=logit[:, :, 1, :],
                                in1=logit[:, :, 0, :], op=ALU.subtract)
        nc.scalar.activation(out=att[:, :, 1, :], in_=att[:, :, 1, :],
                             func=ACTF.Sigmoid)
        nc.vector.tensor_scalar(att[:, :, 0, :], att[:, :, 1, :], -1.0, 1.0,
                                ALU.mult, ALU.add)

        prod2 = sb.tile([T, 2, 2, NUM_HEADS, dh], F32, tag=f"attbuf{l}")
        nc.vector.tensor_tensor(
            out=prod2[:, :, :, :, :],
            in0=vc[:, :, :].rearrange("p t (h d) -> p t h d", d=dh)
                .unsqueeze(1).to_broadcast([T, 2, 2, NUM_HEADS, dh]),
            in1=att[:, :, :, :].unsqueeze(4).to_broadcast([T, 2, 2, NUM_HEADS, dh]),
            op=ALU.mult)
        o_t = sb.tile([T, 2, NUM_HEADS, dh], F32, tag=f"o{l}")
        nc.vector.tensor_reduce(
            out=o_t[:, :, :, :],
            in_=prod2[:, :, :, :, :].transpose([0, 1, 3, 4, 2]),
            axis=mybir.AxisListType.X, op=ALU.add)
        go = sb.tile([T, 2, NUM_HEADS, dh], F32, tag=f"go{l}")
        nc.vector.tensor_tensor(
            out=go[:, :, :, :], in0=o_t[:, :, :, :],
            in1=ga[:, :, :].unsqueeze(3).to_broadcast([T, 2, NUM_HEADS, dh]),
            op=ALU.mult)
        asum = sb.tile([T, C], F32, tag=f"asum{l}")
        nc.vector.tensor_reduce(
            out=asum[:, :],
            in_=go[:, :, :, :].transpose([0, 2, 3, 1]),
            axis=mybir.AxisListType.X, op=ALU.add)

        # ---- (mean_s gated attn) @ wo + bo, accumulated into F_ps
        aT = []
        for ch in range(nch):
            tp = ps2.tile([128, T], F32, tag="kfpt")
            nc.tensor.transpose(out=tp[:], in_=asum[:, ch * 128:(ch + 1) * 128],
                                identity=ident[:T, :T])
            at = sm.tile([128, T], F32R, tag=f"aT{l}_{ch}")
            nc.vector.tensor_copy(at[:], tp[:])
            aT.append(at)
        mm_acc(F_ps[:, :], aT, lambda i: P(f"wo_{l}_{i}"), BR(f"bo_{l}"),
               start=False, stop=True)

        out_sb = sb.tile([T, C], F32, tag=f"out{l}")
        nc.vector.tensor_copy(out_sb[:], F_ps[:, :])
        nc.sync.dma_start(
            f_d[l].ap().rearrange("b n c -> (b n) c"), out_sb[:])


# ------------------------------------------------------------------- driver
_CACHED_NC = None


def kernel(**inputs):
    global _CACHED_NC
    from concourse.bass_utils import run_bass_kernel_spmd

    if _CACHED_NC is None:
        _CACHED_NC = build_program(loop=False)
    nc = _CACHED_NC
    in_maps = prepare_core_inputs(inputs)
    res = run_bass_kernel_spmd(nc, in_maps, list(range(N_CORES)))
    f0 = np.concatenate([res.results[c]["f0"] for c in range(N_CORES)], axis=0)
    f1 = np.concatenate([res.results[c]["f1"] for c in range(N_CORES)], axis=0)
    return f0, f1
